# revision 1
# baseline (speedup 1.0000x reference)
"""GQA attention (B=2, S=2048, HID=2048, 32 q heads / 8 kv heads, fp32 I/O)
on 8 TRN2 NeuronCores.

Sharding: sequence-parallel with fully local K/V. Core c owns 512 query
tokens of batch c//4 (cores 0-3 = batch 0, cores 4-7 = batch 1), but
computes K^T and V for ALL 2048 tokens of its batch locally — that
(+~40% KV projection FLOPs) is much cheaper than an intra-chip
AllGather, which measures 100-170us and blockades the DMA engines while
it runs. Attention is permutation-invariant over keys, so each core
orders tokens own-block-first (host-side permutation) and the device
program stays rank-independent.

V carries a fused ones-column per kv head so the PV matmul also
produces the softmax row-sums; the output-projection bias is fused as
an extra contraction row. All matmuls run in bf16 with fp32 PSUM
accumulation (fp32 matmul is 4x slower on the PE). Heads are processed
in kv-parity pairs whose score matmuls occupy different PE row groups
(concurrent); score PSUM tiles span two key chunks so each Exp covers
N=1024, amortizing ACT's fixed per-instruction overhead. The attention
inner loop is ScalarE(exp)-bound; Q-projection chunks are interleaved
into the pair loop so they run in the PE's slack.

All transposes / casts / padding are done host-side in numpy.
"""

import functools
from contextlib import ExitStack

import numpy as np
import ml_dtypes

import concourse.bass as bass
import concourse.mybir as mybir
import concourse.tile as tile
from concourse import bacc
from concourse.bass_utils import run_bass_kernel_spmd

BF = mybir.dt.bfloat16
F32 = mybir.dt.float32

B, S, HID = 2, 2048, 2048
NH, NKV, HD = 32, 8, 64          # q heads, kv heads, head dim
GRP = NH // NKV                  # 4 q heads per kv head
TP = 4                           # cores per batch group
TOK = S // TP                    # 512 local query tokens per core
KC = HID // 128                  # 16 contraction chunks of 128
NKC = S // 128                   # 16 key chunks of 128 (full seq)
VW = NKV * (HD + 1)              # 520: V width incl. ones columns
EXP_SCALE = float(HD) ** -0.5    # 1/8 softmax scale, fused into Exp


def q_slot(h):
    """qTp tile index and partition base for head h.

    Head h lives at partition base ((h//4)%2)*64 — the same base its kv
    head kh=h//4 occupies inside the kTg tiles, so the scores matmul's
    lhsT and rhs stay partition-aligned.
    """
    return ((h // 4) // 2) * 4 + (h % 4), ((h // 4) % 2) * 64


def build_graph():
    nc = bacc.Bacc(None, target_bir_lowering=False, debug=False, num_devices=8)

    xT = nc.declare_dram_parameter("xT", [HID, S], BF, isOutput=False)
    wkT = nc.declare_dram_parameter("wkT", [HID, NKV * HD], BF, isOutput=False)
    wvT = nc.declare_dram_parameter("wvT", [HID, NKV * HD], BF, isOutput=False)
    wqT = nc.declare_dram_parameter("wqT", [HID, HID], BF, isOutput=False)
    woT = nc.declare_dram_parameter("woT", [HID + 1, HID], BF, isOutput=False)
    out = nc.declare_dram_parameter("out", [TOK, HID], F32, isOutput=True)

    with tile.TileContext(nc) as tc, ExitStack() as es:
        pers = es.enter_context(tc.tile_pool(name="pers", bufs=1))

        def T(shape, dtype, *, name):
            return pers.tile(shape, dtype, name=name, tag=name)

        # ---- SBUF inputs; DMA issue order = priority --------------------
        # xq: own 512 query-token columns (live through attention for the
        # Q projection).  xr/wk/wv live only through phase A (pool xin
        # closes after it, freeing 80KB/partition for attention pools).
        xin_cm = tc.tile_pool(name="xin", bufs=1)
        xin = xin_cm.__enter__()
        xq = [T([128, TOK], BF, name=f"xq{k}") for k in range(KC)]
        xr = [xin.tile([128, S - TOK], BF, tag=f"xr{k}", name=f"xr{k}")
              for k in range(KC)]
        wk_sb = [xin.tile([128, NKV * HD], BF, tag=f"wk{k}", name=f"wk{k}")
                 for k in range(KC)]
        wv_sb = [xin.tile([128, NKV * HD], BF, tag=f"wv{k}", name=f"wv{k}")
                 for k in range(KC)]
        for k in range(KC):
            nc.sync.dma_start(out=xq[k][:, :], in_=xT[k * 128:(k + 1) * 128, 0:TOK])
            nc.sync.dma_start(out=xr[k][:, :], in_=xT[k * 128:(k + 1) * 128, TOK:S])
            nc.sync.dma_start(out=wk_sb[k][:, :], in_=wkT[k * 128:(k + 1) * 128, :])
            nc.sync.dma_start(out=wv_sb[k][:, :], in_=wvT[k * 128:(k + 1) * 128, :])

        def xcols(k, lo, n):
            # columns lo..lo+n of the permuted x^T chunk k
            return xq[k][:, lo:lo + n] if lo < TOK \
                else xr[k][:, lo - TOK:lo - TOK + n]

        # row HD (partition 64) is the K=1 lhsT for the row-sum broadcast
        ones64 = T([HD + 1, 64], BF, name="ones64")
        nc.vector.memset(ones64[:, :], 1.0)
        ones128 = T([1, 128], BF, name="ones128")
        nc.vector.memset(ones128[:, :], 1.0)

        # kTg[nb*4+mt]: [128, 512] = K^T rows mt*128.. for key block nb
        # (kv heads 2mt at partitions 0-63, 2mt+1 at 64-127).
        # vg[c]: [128, 520] V_aug rows for key chunk c, ones at col
        # kh*65+64 of each kv head kh.
        kTg = [T([128, TOK], BF, name=f"kTg{i}") for i in range(16)]
        vg = [T([128, VW], BF, name=f"vg{c}") for c in range(NKC)]
        qTp = [T([128, TOK], BF, name=f"qTp{i}") for i in range(NH // 2)]
        attnT = [T([128, TOK], BF, name=f"attnT{t}") for t in range(NH // 2)]

        # =============== phase A: K^T and V_aug for the whole batch ======
        with tc.tile_pool(name="accA", bufs=3, space="PSUM") as accA:
            for nb in range(TP):
                for mt in range(NKV // 2):
                    ps = accA.tile([128, TOK], F32, tag="acc",
                                   name=f"psk{nb}_{mt}")
                    for k in range(KC):
                        nc.tensor.matmul(
                            out=ps[:, :],
                            lhsT=wk_sb[k][:, mt * 128:(mt + 1) * 128],
                            rhs=xcols(k, nb * TOK, TOK),
                            start=(k == 0), stop=(k == KC - 1))
                    nc.vector.tensor_copy(out=kTg[nb * 4 + mt][:, :],
                                          in_=ps[:, :])
                for tc4 in range(TP):
                    c = nb * 4 + tc4
                    ps = accA.tile([128, NKV * HD], F32, tag="acc",
                                   name=f"psv{c}")
                    for k in range(KC):
                        nc.tensor.matmul(
                            out=ps[:, :],
                            lhsT=xcols(k, c * 128, 128),
                            rhs=wv_sb[k][:, :],
                            start=(k == 0), stop=(k == KC - 1))
                    nc.vector.memset(vg[c][:, :], 1.0)
                    for kh in range(NKV):
                        nc.vector.tensor_copy(
                            out=vg[c][:, kh * (HD + 1):kh * (HD + 1) + HD],
                            in_=ps[:, kh * HD:(kh + 1) * HD])
        xin_cm.__exit__(None, None, None)

        # =============== phases B+D interleaved: Q chunks + attention ====
        # Q chunk m (q dims m*128..) fills heads 2m, 2m+1. Pair group qg
        # (pairs 4qg..4qg+3) needs exactly Q chunks {4qg, 4qg+2, 4qg+1,
        # 4qg+3}, so each group's chunks are emitted right before its
        # pairs and later groups hide in the ACT-bound attention slack.
        pairs = []
        for g in range(0, NKV, 2):
            for j in range(GRP):
                pairs.append((g * GRP + j, (g + 1) * GRP + j))

        def emit_q_chunk(m, wqp, accB, stgB):
            ps = accB.tile([128, TOK], F32, tag="accq", name=f"psq{m}")
            for k in range(KC):
                w = wqp.tile([128, 128], BF, tag="wq", name=f"wq{m}_{k}")
                nc.sync.dma_start(
                    out=w[:, :],
                    in_=wqT[k * 128:(k + 1) * 128, m * 128:(m + 1) * 128])
                nc.tensor.matmul(
                    out=ps[:, :],
                    lhsT=w[:, :],
                    rhs=xq[k][:, :],
                    start=(k == 0), stop=(k == KC - 1))
            st = stgB.tile([128, TOK], BF, tag="stg", name=f"stq{m}")
            nc.vector.tensor_copy(out=st[:, :], in_=ps[:, :])
            # route each head to its kv-parity-aligned slot via DMA
            for j in range(2):
                h = 2 * m + j
                i, roff = q_slot(h)
                nc.sync.dma_start(out=qTp[i][roff:roff + 64, :],
                                  in_=st[j * 64:(j + 1) * 64, :])

        def emit_normalize(h, po, bps, nrm):
            # row HD of po is the softmax denominator; all per-row work
            # stays on partition 64 (DVE can't shift partitions; DMA
            # can't read PSUM).
            lsum = nrm.tile([HD + 1, TOK], F32, tag="lsum", name=f"ls{h}")
            nc.vector.tensor_copy(out=lsum[HD:HD + 1, :], in_=po[HD:HD + 1, :])
            rcp = nrm.tile([HD + 1, TOK], F32, tag="rcp", name=f"rc{h}")
            nc.vector.reciprocal(out=rcp[HD:HD + 1, :], in_=lsum[HD:HD + 1, :])
            rcpb = nrm.tile([HD + 1, TOK], BF, tag="rcpb", name=f"rb{h}")
            nc.vector.tensor_copy(out=rcpb[HD:HD + 1, :], in_=rcp[HD:HD + 1, :])
            pb = bps.tile([64, TOK], F32, tag="pb", name=f"pb{h}")
            nc.tensor.matmul(out=pb[:, :], lhsT=ones64[HD:HD + 1, :],
                             rhs=rcpb[HD:HD + 1, :], start=True, stop=True)
            rb = nrm.tile([64, TOK], BF, tag="rbb", name=f"rbb{h}")
            nc.vector.tensor_copy(out=rb[:, :], in_=pb[:, :])
            ah = nrm.tile([64, TOK], BF, tag="ah", name=f"ah{h}")
            nc.vector.tensor_mul(out=ah[:, :], in0=po[0:HD, :], in1=rb[:, :])
            # place into the pair tile (DMA shifts partitions for odd h)
            t, half = h // 2, (h % 2) * 64
            nc.sync.dma_start(out=attnT[t][half:half + 64, :], in_=ah[:, :])

        # wop opened early so Wo tiles prefetch during attention
        wqp = es.enter_context(tc.tile_pool(name="wqp", bufs=48))
        wop = es.enter_context(tc.tile_pool(name="wop", bufs=16))
        with tc.tile_pool(name="accB", bufs=1, space="PSUM") as accB, \
             tc.tile_pool(name="stgB", bufs=2) as stgB, \
             tc.tile_pool(name="sps", bufs=2, space="PSUM") as sps, \
             tc.tile_pool(name="ops", bufs=2, space="PSUM") as ops, \
             tc.tile_pool(name="bps", bufs=1, space="PSUM") as bps, \
             tc.tile_pool(name="ptp", bufs=4) as ptp, \
             tc.tile_pool(name="nrm", bufs=2) as nrm:
            pending = []    # deferred normalizes: list of (h, po)
            for pi, (hA, hB) in enumerate(pairs):
                if pi % 4 == 0:
                    qg = pi // 4
                    for m in (4 * qg, 4 * qg + 2, 4 * qg + 1, 4 * qg + 3):
                        emit_q_chunk(m, wqp, accB, stgB)
                khA, khB = hA // GRP, hB // GRP
                kt = khA // 2      # kTg row tile: khA at 0:64, khB at 64:128
                qiA, _ = q_slot(hA)
                qiB, _ = q_slot(hB)
                poA = ops.tile([HD + 1, TOK], F32, tag="po", name=f"poA{hA}")
                poB = ops.tile([HD + 1, TOK], F32, tag="po", name=f"poB{hB}")
                pts = []
                for ci in range(0, NKC, 2):
                    psA = sps.tile([128, 2 * TOK], F32, tag="ps",
                                   name=f"psA{hA}_{ci}")
                    psB = sps.tile([128, 2 * TOK], F32, tag="ps",
                                   name=f"psB{hB}_{ci}")
                    for dc in range(2):
                        c = ci + dc
                        nb, lc = c // 4, c % 4
                        kts = kTg[nb * 4 + kt]
                        nc.tensor.matmul(
                            out=psA[:, dc * TOK:(dc + 1) * TOK],
                            lhsT=kts[0:64, lc * 128:(lc + 1) * 128],
                            rhs=qTp[qiA][0:64, :], start=True, stop=True)
                        nc.tensor.matmul(
                            out=psB[:, dc * TOK:(dc + 1) * TOK],
                            lhsT=kts[64:128, lc * 128:(lc + 1) * 128],
                            rhs=qTp[qiB][64:128, :], start=True, stop=True)
                    ptA = ptp.tile([128, 2 * TOK], BF, tag="pt",
                                   name=f"ptA{hA}_{ci}")
                    nc.scalar.activation(
                        out=ptA[:, :], in_=psA[:, :],
                        func=mybir.ActivationFunctionType.Exp, scale=EXP_SCALE)
                    ptB = ptp.tile([128, 2 * TOK], BF, tag="pt",
                                   name=f"ptB{hB}_{ci}")
                    nc.scalar.activation(
                        out=ptB[:, :], in_=psB[:, :],
                        func=mybir.ActivationFunctionType.Exp, scale=EXP_SCALE)
                    pts.append((ci, ptA, ptB))
                # normalizes of the previous pair land here so their PE
                # broadcast matmul never stalls the PE stream
                for h, po in pending:
                    emit_normalize(h, po, bps, nrm)
                pending = [(hA, poA), (hB, poB)]
                for ci, ptA, ptB in pts:
                    for dc in range(2):
                        c = ci + dc
                        nc.tensor.matmul(
                            out=poA[:, :],
                            lhsT=vg[c][:, khA * (HD + 1):(khA + 1) * (HD + 1)],
                            rhs=ptA[:, dc * TOK:(dc + 1) * TOK],
                            start=(c == 0), stop=(c == NKC - 1))
                        nc.tensor.matmul(
                            out=poB[:, :],
                            lhsT=vg[c][:, khB * (HD + 1):(khB + 1) * (HD + 1)],
                            rhs=ptB[:, dc * TOK:(dc + 1) * TOK],
                            start=(c == 0), stop=(c == NKC - 1))
            for h, po in pending:
                emit_normalize(h, po, bps, nrm)

        # =============== phase E: output projection + bias ===========
        with tc.tile_pool(name="yps", bufs=4, space="PSUM") as yps, \
             tc.tile_pool(name="ystg", bufs=3) as ystg:
            for nt in range(4):        # 4 output column blocks of 512
                wo_last = wop.tile([1, 512], BF, tag="wolast",
                                   name=f"wl{nt}")
                nc.sync.dma_start(
                    out=wo_last[:, :],
                    in_=woT[HID:HID + 1, nt * 512:(nt + 1) * 512])
                pys = [yps.tile([128, 512], F32, tag="py",
                                name=f"py{nt}_{i}") for i in range(4)]
                for kc in range(KC):
                    wo_t = wop.tile([128, 512], BF, tag="wo",
                                    name=f"wo{nt}_{kc}")
                    nc.sync.dma_start(
                        out=wo_t[:, :],
                        in_=woT[kc * 128:(kc + 1) * 128,
                                nt * 512:(nt + 1) * 512])
                    for mt in range(4):
                        nc.tensor.matmul(
                            out=pys[mt][:, :],
                            lhsT=attnT[kc][:, mt * 128:(mt + 1) * 128],
                            rhs=wo_t[:, :],
                            start=(kc == 0), stop=False)
                for mt in range(4):    # bias via ones row, K=1 matmul
                    nc.tensor.matmul(
                        out=pys[mt][:, :], lhsT=ones128[:, :],
                        rhs=wo_last[:, :], start=False, stop=True)
                    ys = ystg.tile([128, 512], F32, tag="ys",
                                   name=f"ys{nt}_{mt}")
                    nc.vector.tensor_copy(out=ys[:, :], in_=pys[mt][:, :])
                    nc.sync.dma_start(
                        out=out[mt * 128:(mt + 1) * 128,
                                nt * 512:(nt + 1) * 512],
                        in_=ys[:, :])

    nc.finalize()
    return nc


@functools.lru_cache(maxsize=1)
def _graph():
    return build_graph()


def make_in_maps(x, Wq, Wk, Wv, Wo, bo):
    bf16 = ml_dtypes.bfloat16
    x = np.asarray(x, np.float32)
    wqT = np.ascontiguousarray(np.asarray(Wq, np.float32).T).astype(bf16)
    wkT = np.ascontiguousarray(np.asarray(Wk, np.float32).T).astype(bf16)
    wvT = np.ascontiguousarray(np.asarray(Wv, np.float32).T).astype(bf16)
    woT = np.concatenate(
        [np.asarray(Wo, np.float32).T,
         np.asarray(bo, np.float32)[None, :]], axis=0).astype(bf16)
    woT = np.ascontiguousarray(woT)
    in_maps = []
    for c in range(8):
        b, r = c // TP, c % TP
        # token permutation: own query block first, rest after (attention
        # is permutation-invariant over keys)
        perm = np.r_[r * TOK:(r + 1) * TOK, 0:r * TOK, (r + 1) * TOK:S]
        xT_c = np.ascontiguousarray(x[b].T[:, perm]).astype(bf16)
        in_maps.append(
            {"xT": xT_c, "wqT": wqT, "wkT": wkT, "wvT": wvT, "woT": woT})
    return in_maps


def kernel(x, Wq, Wk, Wv, Wo, bo):
    nc = _graph()
    in_maps = make_in_maps(x, Wq, Wk, Wv, Wo, bo)
    res = run_bass_kernel_spmd(nc, in_maps, core_ids=list(range(8)))
    out = np.empty((B, S, HID), np.float32)
    for c in range(8):
        b, r = c // TP, c % TP
        out[b, r * TOK:(r + 1) * TOK, :] = np.asarray(
            res.results[c]["out"], np.float32)
    return out

